# revision 56
# baseline (speedup 1.0000x reference)
"""ChannelDeconv (training-mode forward, C == block == 64) on 8 TRN2 NeuronCores.

Reference math (see problem):
    x: (32, 64, 128, 128) f32, NCHW
    x1    = x.transpose(1,0,2,3).reshape(64, N*H*W)        # [B, L], L = 524288
    x1_s  = x1[:, ::9]                                     # 58255 sampled cols
    mean  = x1_s.mean(-1)                                  # [B]
    cov   = x1_s @ x1_s.T / n_s + 0.01*I                   # [B, B]
    D     = newton_schulz_isqrt(cov, 5)
    y     = D @ (x1 - mean)  -> reshape back to NCHW

Sharding: data-parallel over N with FULLY REPLICATED statistics -- no
collective. The v1 kernel all-reduced the [65,65] partial Gram; the
framework collective has an ~80us floor (runtime start-alignment +
two-phase CC machinery) that sat square on the critical path. Instead,
every core receives the ENTIRE global stride-9 sample set, quantized to
fp8-e4m3 (3.97MB, ~11us of extra DMA vs ~80us of collective), computes
the full Gram redundantly on its PE, and proceeds with zero cross-core
communication. exec_time is the max of per-core spans, so removing the
rendezvous also removes the dispatch-skew exposure.

Precision budget (host-measured, deterministic for the fixed seed):
fp8 samples for cov/mean + bf16 x1/D for the whitening matmul + bf16
output = 2.7e-3 rel err vs the 2e-2 gate. (Local per-core stats would
avoid replication entirely but measures 2.2e-2 -- over the gate.)

Per core:
  - xp  [128, 32768] bf16: its 4 images, partitions 0:64 = channels of
    images 0,1; 64:128 = channels of images 2,3.
  - xs  [128, 456*72] fp8e4m3: the global sample set, 128 samples per
    chunk (cols 0:64 = channels, col 64 = 1.0 validity, 65:72 pad for
    8B-aligned LDWEIGHTS offsets). All 456 chunk matmuls accumulate
    into ONE [65,65] PSUM tile (cross-chunk summing is free in PSUM;
    the validity column yields the column sums in the same matmul);
    456 x 65 cols ~= 29.6k PE cycles ~= 12us at 2.4GHz. Extraction
    uses single-PSUM-input DVE ops (the walrus verifier rejects
    InstTensorTensor with two PSUM operands).
  - Newton-Schulz runs on M = Graw + n*eps*I (the iterates are
    invariant to the 1/n scale; the final deconv picks up sqrt(n),
    folded into the existing 0.5/sqrt(normA) constant).
  - Whitening: block-diag [[D,0],[0,D]] bf16 matmul over xp, bias
    -D@mean during PSUM eviction (DVE+ACT split), bf16 stores.
"""

import importlib.util
import os
import sys

if importlib.util.find_spec("concourse") is None:
    for _p in ("/opt/trn_rl_repo", os.path.expanduser("~/.axon_site/_ro/trn_rl_repo")):
        if os.path.isdir(_p) and _p not in sys.path:
            sys.path.insert(0, _p)

import numpy as np

N, C, H, W = 32, 64, 128, 128
HW = H * W               # 16384
B = 64                   # whitening block / channel count
STRIDE2 = 9              # sampling stride**2
EPS = 0.01
N_ITER = 5
CORES = 8
NL = N // CORES          # images per core = 4
WIDE = 2 * HW            # stacked free dim = 32768
NS_TOT = (N * HW + STRIDE2 - 1) // STRIDE2   # 58255 global samples
SROWS = 58368            # padded to 456*128
NCH = SROWS // 128       # 456 chunks of 128 samples
CCOL = 66                # 64 channels + 1 validity + 1 pad
RSPAN = 65               # cols actually streamed per chunk
# 12 equal xs tiles, all concurrent. (Both explicit gating -- which demotes
# descriptors to the slow software-DMA path -- and size-shaped descriptor
# priority were tried and measured worse; the plain concurrent pattern wins.)
XS_SIZES = [38] * 12     # chunks, sum 456
XSTILES = len(XS_SIZES)

_cached = {}


def _build_nc():
    import concourse.mybir as mybir
    import concourse.tile as tile
    from concourse import bacc

    f32 = mybir.dt.float32
    bf16 = mybir.dt.bfloat16
    f8 = mybir.dt.float8e4
    nc = bacc.Bacc(None, num_devices=CORES)

    xp = nc.declare_dram_parameter("xp", [128, WIDE], bf16, isOutput=False)
    xs = nc.declare_dram_parameter("xs", [128, NCH * CCOL], f8, isOutput=False)
    eye_in = nc.declare_dram_parameter("eye", [B, B], f32, isOutput=False)
    # bf16 output shaped [8, 128, 4096]: each store is one fully
    # contiguous 1MB DRAM write with 8KB per-partition packets -- DMA
    # bandwidth scales with packet size (4KB packets measured ~170GB/s
    # per descriptor, the store-drain tail was ~8us). Host upcasts.
    NBLK = 512                      # one PSUM bank of f32
    GRP = 8                         # whitening blocks per output DMA
    NJO = WIDE // (NBLK * GRP)      # 8 output stores of 1MB
    out_ext = nc.declare_dram_parameter("out", [NJO, 128, NBLK * GRP], bf16,
                                        isOutput=True)

    with tile.TileContext(nc) as tc:
        with (
            tc.tile_pool(name="big", bufs=1) as big,
            tc.tile_pool(name="stage", bufs=1) as stage,
            tc.tile_pool(name="smalls", bufs=1) as smalls,
            tc.tile_pool(name="nsp", bufs=2) as nsp,
            tc.tile_pool(name="pss", bufs=2, space="PSUM") as pss,
            tc.tile_pool(name="psw", bufs=3, space="PSUM") as psw,
            tc.tile_pool(name="outs", bufs=6) as outs,
        ):
            # ---- input DMAs -------------------------------------------------
            # xs streams FIRST across all three DMA rings (SP/ACT/gpsimd) so
            # the Gram can start ~3us in; S follows, with gpsimd taking the
            # bigger share so the sync+scalar rings drain early for stores.
            # Each ring's 16 HW DMA engines pull queued descriptors
            # CONCURRENTLY, so without explicit deps every xs tile and all of
            # S land near-simultaneously (~12us in) and the Gram sits idle.
            # CHAIN the xs tiles: a tiny DVE copy from tile k into tile k+1's
            # region gives the k+1 DMA a WAW wait -> in-order arrival, tile 0
            # lands ~1us after the preamble and the Gram starts immediately;
            # supply (full HBM rate) stays ahead of the PE's 130cyc/chunk.
            # All DMAs go out concurrently (each ring's 16 HW engines pull
            # queued descriptors in parallel; dependency-gated orderings were
            # tried and interact badly with the DMA scheduler -- chained xs
            # capped single-tile bandwidth, gated S arrived 20us late).
            xs_rings = [nc.sync, nc.scalar, nc.gpsimd]
            xs_tiles = []
            xoff = 0
            for gi, gn in enumerate(XS_SIZES):
                t = stage.tile([128, gn * CCOL], f8, name=f"xs{gi}",
                               tag=f"xs{gi}")
                xs_rings[gi % 3].dma_start(
                    out=t[:, :], in_=xs[:, xoff * CCOL:(xoff + gn) * CCOL])
                xs_tiles.append(t)
                xoff += gn
            eye_sb = smalls.tile([B, B], f32)
            nc.gpsimd.dma_start(out=eye_sb[:, :], in_=eye_in[:, :])
            # prewarm the ACT function tables (Sqrt / Identity): the first use
            # of a table triggers a ~1.3us ACT_TABLE_LOAD, which otherwise
            # lands mid Newton-Schulz on the critical path.
            warm = smalls.tile([B, 3], f32, tag="warm")
            nc.scalar.activation(out=warm[:, 0:1], in_=eye_sb[:, 0:1],
                                 func=mybir.ActivationFunctionType.Sqrt)
            nc.scalar.activation(out=warm[:, 1:2], in_=eye_sb[:, 0:1],
                                 func=mybir.ActivationFunctionType.Identity,
                                 bias=0.0, scale=1.0)

            S = big.tile([128, WIDE], bf16)
            S_SIZES = [4096] * 8     # cols, sum 32768
            s_rings = [nc.sync, nc.scalar, nc.gpsimd]
            soff = 0
            for i, sn in enumerate(S_SIZES):
                s_rings[i % 3].dma_start(
                    out=S[:, soff:soff + sn],
                    in_=xp[:, soff:soff + sn],
                )
                soff += sn

            # dblk allocated up front: the off-diagonal zero quadrants have no
            # data deps, so putting them FIRST in DVE's in-order queue runs
            # them at t~0 instead of behind the whole Newton-Schulz chain.
            dblk = smalls.tile([128, 128], bf16)
            zeros128 = smalls.tile([128, B], bf16)
            nc.vector.memset(zeros128[:, :], 0.0)
            nc.vector.tensor_copy(out=dblk[0:B, B:128], in_=zeros128[0:B, :])
            nc.vector.tensor_copy(out=dblk[B:128, 0:B], in_=zeros128[B:128, :])

            # ---- full Gram (and column sums via the validity column) --------
            # chunk c: g[65,65] += lhsT[128,65]^T @ rhs[128,65]; the
            # validity column doubles as both the sums row/col and count.
            g_ps = pss.tile([RSPAN, RSPAN], f32, tag="pss")
            ci = 0
            for gi, gn in enumerate(XS_SIZES):
                t = xs_tiles[gi]
                for i in range(gn):
                    off = i * CCOL
                    chunk = t[:, off:off + RSPAN]
                    nc.tensor.matmul(
                        g_ps[:, :], lhsT=chunk, rhs=chunk,
                        start=(ci == 0), stop=(ci == NCH - 1),
                    )
                    ci += 1

            # ---- M = Graw + n*eps*I -----------------------------------------
            # (Newton-Schulz iterates are invariant to the 1/n scale; the
            # final deconv regains sqrt(n) via the existing constant.)
            epsn_eye = smalls.tile([B, B], f32, tag="epsn")
            nc.vector.tensor_scalar_mul(out=epsn_eye[:, :], in0=eye_sb[:, :],
                                        scalar1=EPS * NS_TOT)
            cov = smalls.tile([B, B], f32, tag="covm")
            nc.vector.tensor_add(out=cov[:, :], in0=g_ps[0:B, 0:B],
                                 in1=epsn_eye[:, :])
            # raw column sums, copied on ACT (keeps DVE free for the serial
            # normA chain that gates Newton-Schulz)
            msum = smalls.tile([B, 1], f32, tag="msum")
            nc.scalar.activation(out=msum[:, :], in_=g_ps[0:B, B:B + 1],
                                 func=mybir.ActivationFunctionType.Copy,
                                 scale=1.0)

            # ---- normA = ||M||_F and derived constants ----------------------
            # engine discipline: small elementwise/copy ops on DVE, Sqrts on
            # ACT (keeps each consumer under the tiny sync-wait budget).
            # eye3 in bf16 (3.0 exact): the NS matmuls run in bf16, which
            # avoids the f32 LOW_HIGH double-pass (two LDWEIGHTS + two
            # MATMULs per logical matmul) that dominated the NS phase.
            eye3 = smalls.tile([B, B], bf16)
            nc.vector.tensor_scalar_mul(out=eye3[:, :], in0=eye_sb[:, :],
                                        scalar1=3.0)
            # bf16 ones-matmul for the partition reduce: ||M|| sensitivity of
            # the final deconv is ~(1-p5)/2 ~ 6e-4, so bf16 rounding of the
            # row sums (0.4%) perturbs D by ~1e-6 -- noise. Avoids the f32
            # LOW_HIGH double-pass on the serial normA chain.
            ones64 = smalls.tile([B, B], bf16)
            nc.vector.memset(ones64[:, :], 1.0)

            # fused square + row-sum in ONE DVE op (accum_out), then the
            # partition reduce on the PE; rnorm = 1/||M|| comes straight off
            # the PSUM via a single Rsqrt -- the old Sqrt+reciprocal pair is
            # two serial hops longer.
            sq = smalls.tile([B, B], f32)
            rsum = smalls.tile([B, 1], bf16)
            with nc.allow_low_precision(
                    reason="bf16 row-sums perturb deconv by ~1e-6 (normA "
                           "sensitivity ~(1-p5)/2); avoids f32 LOW_HIGH mm"):
                nc.vector.scalar_tensor_tensor(
                    out=sq[:, :], in0=cov[:, :], scalar=1.0, in1=cov[:, :],
                    op0=mybir.AluOpType.mult, op1=mybir.AluOpType.mult,
                    accum_out=rsum[:, :])
            nsq_ps = pss.tile([B, 1], f32, tag="pss")
            nc.tensor.matmul(nsq_ps[:, :], lhsT=ones64[:, :], rhs=rsum[:, :],
                             start=True, stop=True)
            normA = smalls.tile([B, 1], f32)
            nc.scalar.activation(out=normA[:, :], in_=nsq_ps[:, :],
                                 func=mybir.ActivationFunctionType.Sqrt)
            rnorm = smalls.tile([B, 1], f32)
            nc.vector.reciprocal(out=rnorm[:, :], in_=normA[:, :])
            rqnorm = smalls.tile([B, 1], f32)
            nc.scalar.activation(out=rqnorm[:, :], in_=rnorm[:, :],
                                 func=mybir.ActivationFunctionType.Sqrt)
            # 0.5 * sqrt(n) / sqrt(normA): folds the last NS iteration's 0.5
            # AND the M = n*cov rescale into the final deconv scale
            rqnorm_nh = smalls.tile([B, 1], f32)
            nc.scalar.activation(out=rqnorm_nh[:, :], in_=rqnorm[:, :],
                                 func=mybir.ActivationFunctionType.Copy,
                                 scale=0.5 * float(np.sqrt(NS_TOT)))

            # PE pstate warmers: the Tensor engine drops from 2.4GHz to
            # 1.2GHz within a few us of sparse work, and the whole whitening
            # phase then runs at the lower clock. Dummy 512-col matmuls on
            # resident xs data (no readers, no unmet deps -- they can never
            # stall the queue) fill the PE's idle windows through the serial
            # Newton-Schulz chain.
            _warm_n = [0]

            def pe_warm():
                wt = psw.tile([128, NBLK], f32, tag="w",
                              name=f"warm{_warm_n[0]}")
                _warm_n[0] += 1
                nc.tensor.matmul(wt[:, :], lhsT=xs_tiles[0][:, 0:128],
                                 rhs=xs_tiles[0][:, 0:NBLK],
                                 start=True, stop=True)

            # ---- Newton-Schulz (bf16 matmuls, f32 PSUM accumulate) ----------
            # All iterates are symmetric polynomials in M and commute, so
            # both updates come from ONE matmul with stacked rhs U = [Y | Z].
            pe_warm()
            U = nsp.tile([B, 2 * B], bf16, tag="U", name="U0")
            nc.vector.tensor_scalar_mul(out=U[:, 0:B], in0=cov[:, :],
                                        scalar1=rnorm[:, :])
            nc.vector.tensor_copy(out=U[:, B:2 * B], in_=eye_sb[:, :])
            zy_ps = None
            for it in range(N_ITER - 1):
                T = nsp.tile([B, B], bf16, tag="T", name=f"T{it}")
                if it == 0:
                    # Z_0 = I  ->  T = 3I - Y
                    nc.vector.tensor_sub(out=T[:, :], in0=eye3[:, :], in1=U[:, 0:B])
                else:
                    zy_ps = pss.tile([B, B], f32, tag="pss", name=f"zy{it}")
                    nc.tensor.matmul(zy_ps[:, :], lhsT=U[:, B:2 * B],
                                     rhs=U[:, 0:B], start=True, stop=True)
                    nc.vector.tensor_sub(out=T[:, :], in0=eye3[:, :], in1=zy_ps[:, :])
                un_ps = pss.tile([B, 2 * B], f32, tag="pss", name=f"un{it}")
                nc.tensor.matmul(un_ps[:, :], lhsT=T[:, :], rhs=U[:, :],
                                 start=True, stop=True)
                pe_warm()
                Un = nsp.tile([B, 2 * B], bf16, tag="U", name=f"U{it + 1}")
                nc.vector.tensor_scalar_mul(out=Un[:, :], in0=un_ps[:, :],
                                            scalar1=0.5)
                U = Un

            # ---- last iteration, fused with dblk build AND bias -------------
            # un5 = [T5|T5]^T @ [rq*Z4 | Z4@(-rq/n * msum)] lands the
            # STACKED, PRE-SCALED [D; D] in PSUM partitions 0:128 (D and T
            # are symmetric) AND the stacked bias -D@mean in column B of the
            # same matmul: col B = T5 @ Z4 @ (-rq/n)*msum = -(1/n) D @ msum.
            # No partition-shift DMAs and no late f32 dm2 matmul gating the
            # evictions.
            msneg2 = smalls.tile([B, 1], bf16, tag="msneg")
            nc.vector.tensor_scalar(out=msneg2[:, :], in0=msum[:, :],
                                    scalar1=rqnorm_nh[:, :],
                                    scalar2=-1.0 / NS_TOT,
                                    op0=mybir.AluOpType.mult,
                                    op1=mybir.AluOpType.mult)
            w1_ps = pss.tile([B, 1], f32, tag="pss", name="w1")
            nc.tensor.matmul(w1_ps[:, :], lhsT=U[:, B:2 * B], rhs=msneg2[:, :],
                             start=True, stop=True)
            zy5 = pss.tile([B, B], f32, tag="pss", name="zy5")
            nc.tensor.matmul(zy5[:, :], lhsT=U[:, B:2 * B], rhs=U[:, 0:B],
                             start=True, stop=True)
            pe_warm()
            T5 = nsp.tile([B, 2 * B], bf16, tag="T", name="T5")
            nc.vector.tensor_sub(out=T5[:, 0:B], in0=eye3[:, :], in1=zy5[:, :])
            nc.vector.tensor_sub(out=T5[:, B:2 * B], in0=eye3[:, :],
                                 in1=zy5[:, :])
            Zsw = nsp.tile([B, B + 1], bf16, tag="Zs", name="Zsw")
            nc.vector.tensor_scalar_mul(out=Zsw[:, 0:B], in0=U[:, B:2 * B],
                                        scalar1=rqnorm_nh[:, :])
            nc.vector.tensor_copy(out=Zsw[:, B:B + 1], in_=w1_ps[:, :])
            un5 = pss.tile([128, B + 1], f32, tag="pss", name="un5")
            nc.tensor.matmul(un5[:, :], lhsT=T5[:, :], rhs=Zsw[:, :],
                             start=True, stop=True)
            pe_warm()
            pe_warm()

            # ---- diagonal blocks of dblk + bias -----------------------------
            # negdm2 first on DVE (it gates the evictions); dblk quadrants
            # split across DVE and ACT (they gate the whitening matmuls).
            negdm2 = smalls.tile([128, 1], f32)
            nc.vector.tensor_copy(out=negdm2[:, :], in_=un5[:, B:B + 1])
            nc.vector.tensor_copy(out=dblk[0:B, 0:B], in_=un5[0:B, 0:B])
            nc.scalar.activation(out=dblk[B:128, B:128], in_=un5[B:128, 0:B],
                                 func=mybir.ActivationFunctionType.Copy,
                                 scale=1.0)

            # ---- whitening: y = Dblk @ S - dm2 ------------------------------
            # bf16 matmuls stream at 1 row/cycle. Each PSUM tile spans TWO
            # banks (1024 f32 cols, filled by two 512-col matmuls -- a
            # matmul may not cross a bank boundary but compute-engine READS
            # may), so each biased eviction op covers 1024 cols, halving the
            # per-instruction overhead that paced the old 512-col evictions.
            PT = 2 * NBLK
            for jo in range(NJO):
                y_sb = outs.tile([128, NBLK * GRP], bf16, tag="y", name=f"y{jo}")
                for ji in range(GRP // 2):
                    w_ps = psw.tile([128, PT], f32, tag="w", name=f"w{jo}_{ji}")
                    for h in range(2):
                        j = jo * GRP + ji * 2 + h
                        nc.tensor.matmul(
                            w_ps[:, h * NBLK:(h + 1) * NBLK], lhsT=dblk[:, :],
                            rhs=S[:, j * NBLK:(j + 1) * NBLK],
                            start=True, stop=True,
                        )
                    # split the biased PSUM->SBUF eviction across DVE and ACT
                    # (GPSIMD cannot read PSUM on TRN2), aligned to the PSUM
                    # bank boundary: DVE's half waits only matmul h=0 and ACT
                    # waits only h=1, so DVE starts one matmul earlier.
                    HB = NBLK
                    nc.vector.tensor_scalar_add(
                        out=y_sb[:, ji * PT:ji * PT + HB],
                        in0=w_ps[:, 0:HB], scalar1=negdm2[:, :],
                    )
                    nc.scalar.activation(
                        out=y_sb[:, ji * PT + HB:(ji + 1) * PT],
                        in_=w_ps[:, HB:PT],
                        func=mybir.ActivationFunctionType.Identity,
                        bias=negdm2[:, :], scale=1.0,
                    )
                # Stores rotate all three rings (gpsimd is free once S is
                # in). The LAST TWO groups are split across rings in
                # half-size stores so the final drain is ~256KB not ~512KB.
                # (Splitting EVERY store was tried: the doubled descriptor
                # count regressed the drain by ~8us.)
                if jo < NJO - 1:
                    eng = [nc.scalar, nc.sync, nc.gpsimd][jo % 3]
                    eng.dma_start(out=out_ext[jo, :, :], in_=y_sb[:, :])
                else:
                    # final group split across two rings (2KB packets are
                    # slower per byte, but halving the LAST transfer wins
                    # on the drain the exec span waits for)
                    HF = NBLK * GRP // 2
                    nc.scalar.dma_start(out=out_ext[jo, :, 0:HF],
                                        in_=y_sb[:, 0:HF])
                    nc.sync.dma_start(out=out_ext[jo, :, HF:],
                                      in_=y_sb[:, HF:])

    nc.finalize()
    return nc


def _shard_inputs(x):
    """Build per-core input maps from the full (32,64,128,128) f32 tensor."""
    import ml_dtypes
    bf16 = ml_dtypes.bfloat16
    f8 = ml_dtypes.float8_e4m3

    x = np.ascontiguousarray(x, dtype=np.float32)

    # global stride-9 sample gather, [n_samples, 64], RAW values (fp8 has
    # no headroom for pre-scaling; 1/n is folded on-device instead)
    xr = x.reshape(N, C, HW)
    ls = np.arange(0, N * HW, STRIDE2, dtype=np.int64)
    xs_all = xr[ls // HW, :, ls % HW]               # [58255, 64]
    xs_pad = np.zeros((SROWS, B), dtype=np.float32)
    xs_pad[:NS_TOT] = xs_all
    # chunk c, partition p = sample 128c+p: [channels | validity 1.0 | pad]
    arr = np.zeros((NCH, 128, CCOL), dtype=f8)
    arr[:, :, 0:B] = xs_pad.reshape(NCH, 128, B).astype(f8)
    arr[:, :, B] = np.float32(1.0)
    xs_packed = np.ascontiguousarray(
        arr.transpose(1, 0, 2).reshape(128, NCH * CCOL))

    eye = np.eye(B, dtype=np.float32)
    in_maps = []
    for k in range(CORES):
        # stacked main input [128, 32768] bf16
        x4 = x[NL * k:NL * (k + 1)].reshape(2, 2, C, HW)
        xp = np.ascontiguousarray(
            x4.transpose(0, 2, 1, 3).reshape(128, WIDE).astype(bf16))
        in_maps.append({"xp": xp, "xs": xs_packed, "eye": eye})
    return in_maps


def _unshard_output(results):
    y = np.empty((N, C, H, W), dtype=np.float32)
    for k in range(CORES):
        o = np.asarray(results[k]["out"]).astype(np.float32)
        o = o.reshape(8, 128, 4096).transpose(1, 0, 2).reshape(128, WIDE)
        o = o.reshape(2, C, 2, HW)
        y[NL * k:NL * (k + 1)] = (
            o.transpose(0, 2, 1, 3).reshape(NL, C, H, W))
    return y


def kernel(x):
    from concourse.bass_utils import run_bass_kernel_spmd

    if "nc" not in _cached:
        _cached["nc"] = _build_nc()
    nc = _cached["nc"]

    in_maps = _shard_inputs(np.asarray(x))
    res = run_bass_kernel_spmd(nc, in_maps, core_ids=list(range(CORES)))
    _cached["last_results"] = res
    return _unshard_output(res.results)


# revision 59
# speedup vs baseline: 1.0487x; 1.0487x over previous
"""ChannelDeconv (training-mode forward, C == block == 64) on 8 TRN2 NeuronCores.

Reference math (see problem):
    x: (32, 64, 128, 128) f32, NCHW
    x1    = x.transpose(1,0,2,3).reshape(64, N*H*W)        # [B, L], L = 524288
    x1_s  = x1[:, ::9]                                     # 58255 sampled cols
    mean  = x1_s.mean(-1)                                  # [B]
    cov   = x1_s @ x1_s.T / n_s + 0.01*I                   # [B, B]
    D     = newton_schulz_isqrt(cov, 5)
    y     = D @ (x1 - mean)  -> reshape back to NCHW

Sharding: data-parallel over N with FULLY REPLICATED statistics -- no
collective. The v1 kernel all-reduced the [65,65] partial Gram; the
framework collective has an ~80us floor (runtime start-alignment +
two-phase CC machinery) that sat square on the critical path. Instead,
every core receives the ENTIRE global stride-9 sample set, quantized to
fp8-e4m3 (3.97MB, ~11us of extra DMA vs ~80us of collective), computes
the full Gram redundantly on its PE, and proceeds with zero cross-core
communication. exec_time is the max of per-core spans, so removing the
rendezvous also removes the dispatch-skew exposure.

Precision budget (host-measured, deterministic for the fixed seed):
fp8 samples for cov/mean + bf16 x1/D for the whitening matmul + bf16
output = 2.7e-3 rel err vs the 2e-2 gate. (Local per-core stats would
avoid replication entirely but measures 2.2e-2 -- over the gate.)

Per core:
  - xp  [128, 32768] bf16: its 4 images, partitions 0:64 = channels of
    images 0,1; 64:128 = channels of images 2,3.
  - xs  [128, 456*72] fp8e4m3: the global sample set, 128 samples per
    chunk (cols 0:64 = channels, col 64 = 1.0 validity, 65:72 pad for
    8B-aligned LDWEIGHTS offsets). All 456 chunk matmuls accumulate
    into ONE [65,65] PSUM tile (cross-chunk summing is free in PSUM;
    the validity column yields the column sums in the same matmul);
    456 x 65 cols ~= 29.6k PE cycles ~= 12us at 2.4GHz. Extraction
    uses single-PSUM-input DVE ops (the walrus verifier rejects
    InstTensorTensor with two PSUM operands).
  - Newton-Schulz runs on M = Graw + n*eps*I (the iterates are
    invariant to the 1/n scale; the final deconv picks up sqrt(n),
    folded into the existing 0.5/sqrt(normA) constant).
  - Whitening: block-diag [[D,0],[0,D]] bf16 matmul over xp, bias
    -D@mean during PSUM eviction (DVE+ACT split), bf16 stores.
"""

import importlib.util
import os
import sys

if importlib.util.find_spec("concourse") is None:
    for _p in ("/opt/trn_rl_repo", os.path.expanduser("~/.axon_site/_ro/trn_rl_repo")):
        if os.path.isdir(_p) and _p not in sys.path:
            sys.path.insert(0, _p)

import numpy as np

N, C, H, W = 32, 64, 128, 128
HW = H * W               # 16384
B = 64                   # whitening block / channel count
STRIDE2 = 9              # sampling stride**2
EPS = 0.01
N_ITER = 5
CORES = 8
NL = N // CORES          # images per core = 4
WIDE = 2 * HW            # stacked free dim = 32768
NS_TOT = (N * HW + STRIDE2 - 1) // STRIDE2   # 58255 global samples
SROWS = 58368            # padded to 456*128
NCH = SROWS // 128       # 456 chunks of 128 samples
CCOL = 66                # 64 channels + 1 validity + 1 pad
RSPAN = 65               # cols actually streamed per chunk
# 12 equal xs tiles, all concurrent. (Both explicit gating -- which demotes
# descriptors to the slow software-DMA path -- and size-shaped descriptor
# priority were tried and measured worse; the plain concurrent pattern wins.)
XS_SIZES = [38] * 12     # chunks, sum 456
XSTILES = len(XS_SIZES)

_cached = {}


def _build_nc():
    import concourse.mybir as mybir
    import concourse.tile as tile
    from concourse import bacc

    f32 = mybir.dt.float32
    bf16 = mybir.dt.bfloat16
    f8 = mybir.dt.float8e4
    nc = bacc.Bacc(None, num_devices=CORES)

    xp = nc.declare_dram_parameter("xp", [128, WIDE], bf16, isOutput=False)
    xs = nc.declare_dram_parameter("xs", [128, NCH * CCOL], f8, isOutput=False)
    eye_in = nc.declare_dram_parameter("eye", [B, B], f32, isOutput=False)
    # bf16 output shaped [16, 128, 2048]: each store is one fully
    # contiguous 512KB DRAM write. Host upcasts back to f32. (1MB stores
    # with 8KB packets were tried: per-engine DMA rate is ~21GB/s
    # regardless of packet size, so the ~9us drain tail is invariant.)
    NBLK = 512                      # one PSUM bank of f32
    GRP = 4                         # whitening blocks per output DMA
    NJO = WIDE // (NBLK * GRP)      # 16 output stores of 512KB
    out_ext = nc.declare_dram_parameter("out", [NJO, 128, NBLK * GRP], bf16,
                                        isOutput=True)

    with tile.TileContext(nc) as tc:
        with (
            tc.tile_pool(name="big", bufs=1) as big,
            tc.tile_pool(name="stage", bufs=1) as stage,
            tc.tile_pool(name="smalls", bufs=1) as smalls,
            tc.tile_pool(name="nsp", bufs=2) as nsp,
            tc.tile_pool(name="pss", bufs=2, space="PSUM") as pss,
            tc.tile_pool(name="psw", bufs=3, space="PSUM") as psw,
            tc.tile_pool(name="outs", bufs=6) as outs,
        ):
            # ---- input DMAs -------------------------------------------------
            # xs streams FIRST across all three DMA rings (SP/ACT/gpsimd) so
            # the Gram can start ~3us in; S follows, with gpsimd taking the
            # bigger share so the sync+scalar rings drain early for stores.
            # Each ring's 16 HW DMA engines pull queued descriptors
            # CONCURRENTLY, so without explicit deps every xs tile and all of
            # S land near-simultaneously (~12us in) and the Gram sits idle.
            # CHAIN the xs tiles: a tiny DVE copy from tile k into tile k+1's
            # region gives the k+1 DMA a WAW wait -> in-order arrival, tile 0
            # lands ~1us after the preamble and the Gram starts immediately;
            # supply (full HBM rate) stays ahead of the PE's 130cyc/chunk.
            # All DMAs go out concurrently (each ring's 16 HW engines pull
            # queued descriptors in parallel; dependency-gated orderings were
            # tried and interact badly with the DMA scheduler -- chained xs
            # capped single-tile bandwidth, gated S arrived 20us late).
            xs_rings = [nc.sync, nc.scalar, nc.gpsimd]
            xs_tiles = []
            xoff = 0
            for gi, gn in enumerate(XS_SIZES):
                t = stage.tile([128, gn * CCOL], f8, name=f"xs{gi}",
                               tag=f"xs{gi}")
                xs_rings[gi % 3].dma_start(
                    out=t[:, :], in_=xs[:, xoff * CCOL:(xoff + gn) * CCOL])
                xs_tiles.append(t)
                xoff += gn
            eye_sb = smalls.tile([B, B], f32)
            nc.gpsimd.dma_start(out=eye_sb[:, :], in_=eye_in[:, :])
            # prewarm the ACT function tables (Sqrt / Identity): the first use
            # of a table triggers a ~1.3us ACT_TABLE_LOAD, which otherwise
            # lands mid Newton-Schulz on the critical path.
            warm = smalls.tile([B, 3], f32, tag="warm")
            nc.scalar.activation(out=warm[:, 0:1], in_=eye_sb[:, 0:1],
                                 func=mybir.ActivationFunctionType.Sqrt)
            nc.scalar.activation(out=warm[:, 1:2], in_=eye_sb[:, 0:1],
                                 func=mybir.ActivationFunctionType.Identity,
                                 bias=0.0, scale=1.0)

            S = big.tile([128, WIDE], bf16)
            S_SIZES = [4096] * 8     # cols, sum 32768
            s_rings = [nc.sync, nc.scalar, nc.gpsimd]
            soff = 0
            for i, sn in enumerate(S_SIZES):
                s_rings[i % 3].dma_start(
                    out=S[:, soff:soff + sn],
                    in_=xp[:, soff:soff + sn],
                )
                soff += sn

            # dblk allocated up front: the off-diagonal zero quadrants have no
            # data deps, so putting them FIRST in DVE's in-order queue runs
            # them at t~0 instead of behind the whole Newton-Schulz chain.
            dblk = smalls.tile([128, 128], bf16)
            zeros128 = smalls.tile([128, B], bf16)
            nc.vector.memset(zeros128[:, :], 0.0)
            nc.vector.tensor_copy(out=dblk[0:B, B:128], in_=zeros128[0:B, :])
            nc.vector.tensor_copy(out=dblk[B:128, 0:B], in_=zeros128[B:128, :])

            # ---- full Gram (and column sums via the validity column) --------
            # chunk c: g[65,65] += lhsT[128,65]^T @ rhs[128,65]; the
            # validity column doubles as both the sums row/col and count.
            g_ps = pss.tile([RSPAN, RSPAN], f32, tag="pss")
            ci = 0
            for gi, gn in enumerate(XS_SIZES):
                t = xs_tiles[gi]
                for i in range(gn):
                    off = i * CCOL
                    chunk = t[:, off:off + RSPAN]
                    nc.tensor.matmul(
                        g_ps[:, :], lhsT=chunk, rhs=chunk,
                        start=(ci == 0), stop=(ci == NCH - 1),
                    )
                    ci += 1

            # ---- M = Graw + n*eps*I -----------------------------------------
            # (Newton-Schulz iterates are invariant to the 1/n scale; the
            # final deconv regains sqrt(n) via the existing constant.)
            epsn_eye = smalls.tile([B, B], f32, tag="epsn")
            nc.vector.tensor_scalar_mul(out=epsn_eye[:, :], in0=eye_sb[:, :],
                                        scalar1=EPS * NS_TOT)
            cov = smalls.tile([B, B], f32, tag="covm")
            nc.vector.tensor_add(out=cov[:, :], in0=g_ps[0:B, 0:B],
                                 in1=epsn_eye[:, :])
            # raw column sums, copied on ACT (keeps DVE free for the serial
            # normA chain that gates Newton-Schulz)
            msum = smalls.tile([B, 1], f32, tag="msum")
            nc.scalar.activation(out=msum[:, :], in_=g_ps[0:B, B:B + 1],
                                 func=mybir.ActivationFunctionType.Copy,
                                 scale=1.0)

            # ---- normA = ||M||_F and derived constants ----------------------
            # engine discipline: small elementwise/copy ops on DVE, Sqrts on
            # ACT (keeps each consumer under the tiny sync-wait budget).
            # eye3 in bf16 (3.0 exact): the NS matmuls run in bf16, which
            # avoids the f32 LOW_HIGH double-pass (two LDWEIGHTS + two
            # MATMULs per logical matmul) that dominated the NS phase.
            eye3 = smalls.tile([B, B], bf16)
            nc.vector.tensor_scalar_mul(out=eye3[:, :], in0=eye_sb[:, :],
                                        scalar1=3.0)
            # bf16 ones-matmul for the partition reduce: ||M|| sensitivity of
            # the final deconv is ~(1-p5)/2 ~ 6e-4, so bf16 rounding of the
            # row sums (0.4%) perturbs D by ~1e-6 -- noise. Avoids the f32
            # LOW_HIGH double-pass on the serial normA chain.
            ones64 = smalls.tile([B, B], bf16)
            nc.vector.memset(ones64[:, :], 1.0)

            # fused square + row-sum in ONE DVE op (accum_out), then the
            # partition reduce on the PE; rnorm = 1/||M|| comes straight off
            # the PSUM via a single Rsqrt -- the old Sqrt+reciprocal pair is
            # two serial hops longer.
            sq = smalls.tile([B, B], f32)
            rsum = smalls.tile([B, 1], bf16)
            with nc.allow_low_precision(
                    reason="bf16 row-sums perturb deconv by ~1e-6 (normA "
                           "sensitivity ~(1-p5)/2); avoids f32 LOW_HIGH mm"):
                nc.vector.scalar_tensor_tensor(
                    out=sq[:, :], in0=cov[:, :], scalar=1.0, in1=cov[:, :],
                    op0=mybir.AluOpType.mult, op1=mybir.AluOpType.mult,
                    accum_out=rsum[:, :])
            nsq_ps = pss.tile([B, 1], f32, tag="pss")
            nc.tensor.matmul(nsq_ps[:, :], lhsT=ones64[:, :], rhs=rsum[:, :],
                             start=True, stop=True)
            normA = smalls.tile([B, 1], f32)
            nc.scalar.activation(out=normA[:, :], in_=nsq_ps[:, :],
                                 func=mybir.ActivationFunctionType.Sqrt)
            rnorm = smalls.tile([B, 1], f32)
            nc.vector.reciprocal(out=rnorm[:, :], in_=normA[:, :])
            rqnorm = smalls.tile([B, 1], f32)
            nc.scalar.activation(out=rqnorm[:, :], in_=rnorm[:, :],
                                 func=mybir.ActivationFunctionType.Sqrt)
            # 0.5 * sqrt(n) / sqrt(normA): folds the last NS iteration's 0.5
            # AND the M = n*cov rescale into the final deconv scale
            rqnorm_nh = smalls.tile([B, 1], f32)
            nc.scalar.activation(out=rqnorm_nh[:, :], in_=rqnorm[:, :],
                                 func=mybir.ActivationFunctionType.Copy,
                                 scale=0.5 * float(np.sqrt(NS_TOT)))

            # PE pstate warmers: the Tensor engine drops from 2.4GHz to
            # 1.2GHz within a few us of sparse work, and the whole whitening
            # phase then runs at the lower clock. Dummy 512-col matmuls on
            # resident xs data (no readers, no unmet deps -- they can never
            # stall the queue) fill the PE's idle windows through the serial
            # Newton-Schulz chain.
            _warm_n = [0]

            def pe_warm():
                wt = psw.tile([128, NBLK], f32, tag="w",
                              name=f"warm{_warm_n[0]}")
                _warm_n[0] += 1
                nc.tensor.matmul(wt[:, :], lhsT=xs_tiles[0][:, 0:128],
                                 rhs=xs_tiles[0][:, 0:NBLK],
                                 start=True, stop=True)

            # ---- Newton-Schulz (bf16 matmuls, f32 PSUM accumulate) ----------
            # All iterates are symmetric polynomials in M and commute, so
            # both updates come from ONE matmul with stacked rhs U = [Y | Z].
            pe_warm()
            U = nsp.tile([B, 2 * B], bf16, tag="U", name="U0")
            nc.vector.tensor_scalar_mul(out=U[:, 0:B], in0=cov[:, :],
                                        scalar1=rnorm[:, :])
            nc.vector.tensor_copy(out=U[:, B:2 * B], in_=eye_sb[:, :])
            zy_ps = None
            for it in range(N_ITER - 1):
                T = nsp.tile([B, B], bf16, tag="T", name=f"T{it}")
                if it == 0:
                    # Z_0 = I  ->  T = 3I - Y
                    nc.vector.tensor_sub(out=T[:, :], in0=eye3[:, :], in1=U[:, 0:B])
                else:
                    zy_ps = pss.tile([B, B], f32, tag="pss", name=f"zy{it}")
                    nc.tensor.matmul(zy_ps[:, :], lhsT=U[:, B:2 * B],
                                     rhs=U[:, 0:B], start=True, stop=True)
                    nc.vector.tensor_sub(out=T[:, :], in0=eye3[:, :], in1=zy_ps[:, :])
                un_ps = pss.tile([B, 2 * B], f32, tag="pss", name=f"un{it}")
                nc.tensor.matmul(un_ps[:, :], lhsT=T[:, :], rhs=U[:, :],
                                 start=True, stop=True)
                pe_warm()
                Un = nsp.tile([B, 2 * B], bf16, tag="U", name=f"U{it + 1}")
                nc.vector.tensor_scalar_mul(out=Un[:, :], in0=un_ps[:, :],
                                            scalar1=0.5)
                U = Un

            # ---- last iteration, fused with dblk build AND bias -------------
            # un5 = [T5|T5]^T @ [rq*Z4 | Z4@(-rq/n * msum)] lands the
            # STACKED, PRE-SCALED [D; D] in PSUM partitions 0:128 (D and T
            # are symmetric) AND the stacked bias -D@mean in column B of the
            # same matmul: col B = T5 @ Z4 @ (-rq/n)*msum = -(1/n) D @ msum.
            # No partition-shift DMAs and no late f32 dm2 matmul gating the
            # evictions.
            msneg2 = smalls.tile([B, 1], bf16, tag="msneg")
            nc.vector.tensor_scalar(out=msneg2[:, :], in0=msum[:, :],
                                    scalar1=rqnorm_nh[:, :],
                                    scalar2=-1.0 / NS_TOT,
                                    op0=mybir.AluOpType.mult,
                                    op1=mybir.AluOpType.mult)
            w1_ps = pss.tile([B, 1], f32, tag="pss", name="w1")
            nc.tensor.matmul(w1_ps[:, :], lhsT=U[:, B:2 * B], rhs=msneg2[:, :],
                             start=True, stop=True)
            zy5 = pss.tile([B, B], f32, tag="pss", name="zy5")
            nc.tensor.matmul(zy5[:, :], lhsT=U[:, B:2 * B], rhs=U[:, 0:B],
                             start=True, stop=True)
            pe_warm()
            T5 = nsp.tile([B, 2 * B], bf16, tag="T", name="T5")
            nc.vector.tensor_sub(out=T5[:, 0:B], in0=eye3[:, :], in1=zy5[:, :])
            nc.vector.tensor_sub(out=T5[:, B:2 * B], in0=eye3[:, :],
                                 in1=zy5[:, :])
            Zsw = nsp.tile([B, B + 1], bf16, tag="Zs", name="Zsw")
            nc.vector.tensor_scalar_mul(out=Zsw[:, 0:B], in0=U[:, B:2 * B],
                                        scalar1=rqnorm_nh[:, :])
            nc.vector.tensor_copy(out=Zsw[:, B:B + 1], in_=w1_ps[:, :])
            un5 = pss.tile([128, B + 1], f32, tag="pss", name="un5")
            nc.tensor.matmul(un5[:, :], lhsT=T5[:, :], rhs=Zsw[:, :],
                             start=True, stop=True)
            pe_warm()
            pe_warm()

            # ---- diagonal blocks of dblk + bias -----------------------------
            # negdm2 first on DVE (it gates the evictions); dblk quadrants
            # split across DVE and ACT (they gate the whitening matmuls).
            negdm2 = smalls.tile([128, 1], f32)
            nc.vector.tensor_copy(out=negdm2[:, :], in_=un5[:, B:B + 1])
            nc.vector.tensor_copy(out=dblk[0:B, 0:B], in_=un5[0:B, 0:B])
            nc.scalar.activation(out=dblk[B:128, B:128], in_=un5[B:128, 0:B],
                                 func=mybir.ActivationFunctionType.Copy,
                                 scale=1.0)

            # ---- whitening: y = Dblk @ S - dm2 ------------------------------
            # bf16 matmuls stream at 1 row/cycle. Each PSUM tile spans TWO
            # banks (1024 f32 cols, filled by two 512-col matmuls -- a
            # matmul may not cross a bank boundary but compute-engine READS
            # may), so each biased eviction op covers 1024 cols, halving the
            # per-instruction overhead that paced the old 512-col evictions.
            PT = 2 * NBLK
            for jo in range(NJO):
                y_sb = outs.tile([128, NBLK * GRP], bf16, tag="y", name=f"y{jo}")
                for ji in range(GRP // 2):
                    w_ps = psw.tile([128, PT], f32, tag="w", name=f"w{jo}_{ji}")
                    for h in range(2):
                        j = jo * GRP + ji * 2 + h
                        nc.tensor.matmul(
                            w_ps[:, h * NBLK:(h + 1) * NBLK], lhsT=dblk[:, :],
                            rhs=S[:, j * NBLK:(j + 1) * NBLK],
                            start=True, stop=True,
                        )
                    # split the biased PSUM->SBUF eviction across DVE and ACT
                    # (GPSIMD cannot read PSUM on TRN2), aligned to the PSUM
                    # bank boundary: DVE's half waits only matmul h=0 and ACT
                    # waits only h=1, so DVE starts one matmul earlier.
                    HB = NBLK
                    nc.vector.tensor_scalar_add(
                        out=y_sb[:, ji * PT:ji * PT + HB],
                        in0=w_ps[:, 0:HB], scalar1=negdm2[:, :],
                    )
                    nc.scalar.activation(
                        out=y_sb[:, ji * PT + HB:(ji + 1) * PT],
                        in_=w_ps[:, HB:PT],
                        func=mybir.ActivationFunctionType.Identity,
                        bias=negdm2[:, :], scale=1.0,
                    )
                # Stores rotate all three rings (gpsimd is free once S is
                # in). The LAST TWO groups are split across rings in
                # half-size stores so the final drain is ~256KB not ~512KB.
                # (Splitting EVERY store was tried: the doubled descriptor
                # count regressed the drain by ~8us.)
                if jo < NJO - 4:
                    eng = [nc.scalar, nc.sync, nc.gpsimd][jo % 3]
                    eng.dma_start(out=out_ext[jo, :, :], in_=y_sb[:, :])
                else:
                    HF = NBLK * GRP // 2
                    e0, e1 = (nc.scalar, nc.sync) if jo % 2 == 0 else \
                        (nc.gpsimd, nc.scalar)
                    e0.dma_start(out=out_ext[jo, :, 0:HF], in_=y_sb[:, 0:HF])
                    e1.dma_start(out=out_ext[jo, :, HF:], in_=y_sb[:, HF:])

    nc.finalize()
    return nc


def _shard_inputs(x):
    """Build per-core input maps from the full (32,64,128,128) f32 tensor."""
    import ml_dtypes
    bf16 = ml_dtypes.bfloat16
    f8 = ml_dtypes.float8_e4m3

    x = np.ascontiguousarray(x, dtype=np.float32)

    # global stride-9 sample gather, [n_samples, 64], RAW values (fp8 has
    # no headroom for pre-scaling; 1/n is folded on-device instead)
    xr = x.reshape(N, C, HW)
    ls = np.arange(0, N * HW, STRIDE2, dtype=np.int64)
    xs_all = xr[ls // HW, :, ls % HW]               # [58255, 64]
    xs_pad = np.zeros((SROWS, B), dtype=np.float32)
    xs_pad[:NS_TOT] = xs_all
    # chunk c, partition p = sample 128c+p: [channels | validity 1.0 | pad]
    arr = np.zeros((NCH, 128, CCOL), dtype=f8)
    arr[:, :, 0:B] = xs_pad.reshape(NCH, 128, B).astype(f8)
    arr[:, :, B] = np.float32(1.0)
    xs_packed = np.ascontiguousarray(
        arr.transpose(1, 0, 2).reshape(128, NCH * CCOL))

    eye = np.eye(B, dtype=np.float32)
    in_maps = []
    for k in range(CORES):
        # stacked main input [128, 32768] bf16
        x4 = x[NL * k:NL * (k + 1)].reshape(2, 2, C, HW)
        xp = np.ascontiguousarray(
            x4.transpose(0, 2, 1, 3).reshape(128, WIDE).astype(bf16))
        in_maps.append({"xp": xp, "xs": xs_packed, "eye": eye})
    return in_maps


def _unshard_output(results):
    y = np.empty((N, C, H, W), dtype=np.float32)
    for k in range(CORES):
        o = np.asarray(results[k]["out"]).astype(np.float32)
        o = o.reshape(16, 128, 2048).transpose(1, 0, 2).reshape(128, WIDE)
        o = o.reshape(2, C, 2, HW)
        y[NL * k:NL * (k + 1)] = (
            o.transpose(0, 2, 1, 3).reshape(NL, C, H, W))
    return y


def kernel(x):
    from concourse.bass_utils import run_bass_kernel_spmd

    if "nc" not in _cached:
        _cached["nc"] = _build_nc()
    nc = _cached["nc"]

    in_maps = _shard_inputs(np.asarray(x))
    res = run_bass_kernel_spmd(nc, in_maps, core_ids=list(range(CORES)))
    _cached["last_results"] = res
    return _unshard_output(res.results)
